# revision 1
# baseline (speedup 1.0000x reference)
"""Banded-DTW 1-NN (KnnDtw) Trainium2 Bass kernel.

Algorithm
---------
Reference computes, per (query q, fit row f), a Sakoe-Chiba banded DTW
(w=10) over length-256 sequences and returns fit_labels[argmin_f dm[q,f]].

Device mapping: in band ("buffer") coordinates, row i keeps 21 cells
c in [0,20] with j = i-10+c.  The update is
    a[c]   = min(prev[c], prev[c+1])
    row[c] = min(a[c], row[c-1]) + |samples[q,i] - fit[f, i-10+c]|
which is exactly one `tensor_tensor_scan` (op0=min, op1=add) per DTW step,
plus one `tensor_tensor` min and one scalar-engine Abs per step.
Out-of-range fit positions are padded with LARGE so band edges fall out
automatically; a -LARGE/+2*LARGE guard element between per-pair segments
resets the scan carry, letting one scan instruction process 32 independent
(q, f) pairs per partition.

Sharding: queries are split across 8 cores (16 each).  Per core the
4096 (q,f) pairs sit on 128 partitions x 32 segments.  Partition
p = q_local*8 + f_hi, segment s -> f = f_hi*32 + s, so the per-partition
activation bias (-samples[q, i]) is constant per partition.

The device returns dm (cost[255,255] per pair); the host does the final
argmin + label gather (trivial, exact).
"""

import numpy as np

import concourse.bass as bass
import concourse.bacc as bacc
import concourse.mybir as mybir
from concourse.tile import TileContext
from concourse import bass_utils

# Problem shapes (hardcoded per harness contract)
NQ, M = 128, 256      # samples
NF, N = 256, 256      # fit_data
NCORES = 8
QPC = NQ // NCORES    # 16 queries per core
CELLS = 21            # band cells per row, c in [0,20], j = i-10+c
SEG = CELLS + 1       # +1 guard element that resets the scan carry
NSEG = 32             # segments (f_lo values) per partition
FD = NSEG * SEG       # 704 scan elements per partition
PAD = 16              # fit row padding on each side
PADF = N + 2 * PAD    # 288
LARGE = np.float32(1e15)
F32 = mybir.dt.float32

_CACHE: dict = {}


def _build_nc() -> bass.Bass:
    nc = bacc.Bacc(
        "TRN2", target_bir_lowering=False, debug=False, num_devices=NCORES
    )

    fit_in = nc.dram_tensor("fit_rep", [128, NSEG * PADF], F32, kind="ExternalInput")
    nsamp_in = nc.dram_tensor("neg_samp", [128, M], F32, kind="ExternalInput")
    row0_in = nc.dram_tensor("row0", [128, FD], F32, kind="ExternalInput")
    atmpl_in = nc.dram_tensor("a_tmpl", [128, FD], F32, kind="ExternalInput")
    dtmpl_in = nc.dram_tensor("d_tmpl", [128, FD], F32, kind="ExternalInput")
    dm_out = nc.dram_tensor("dm_out", [128, NSEG], F32, kind="ExternalOutput")

    amin = mybir.AluOpType.min
    aadd = mybir.AluOpType.add
    fabs = mybir.ActivationFunctionType.Abs

    with TileContext(nc) as tc:
        with tc.tile_pool(name="main", bufs=1) as pool:
            fit_sb = pool.tile([128, NSEG * PADF], F32)
            nsamp = pool.tile([128, M], F32)
            row_a = pool.tile([128, FD], F32)
            row_b = pool.tile([128, FD], F32)
            a_arr = pool.tile([128, FD], F32)
            d_a = pool.tile([128, FD], F32)
            d_b = pool.tile([128, FD], F32)
            dmc = pool.tile([128, NSEG], F32)

            nc.sync.dma_start(out=fit_sb[:], in_=fit_in[:, :])
            nc.sync.dma_start(out=nsamp[:], in_=nsamp_in[:, :])
            nc.sync.dma_start(out=row_a[:], in_=row0_in[:, :])
            nc.sync.dma_start(out=a_arr[:], in_=atmpl_in[:, :])
            nc.sync.dma_start(out=d_a[:], in_=dtmpl_in[:, :])
            nc.sync.dma_start(out=d_b[:], in_=dtmpl_in[:, :])

            fit3 = fit_sb.rearrange("p (s c) -> p s c", c=PADF)
            a3 = a_arr.rearrange("p (s c) -> p s c", c=SEG)
            rows = [row_a, row_b]
            rows3 = [r.rearrange("p (s c) -> p s c", c=SEG) for r in rows]
            ds = [d_a, d_b]
            ds3 = [d.rearrange("p (s c) -> p s c", c=SEG) for d in ds]

            for i in range(1, M):
                rin3 = rows3[(i - 1) % 2]
                rout = rows[i % 2]
                dt = ds[i % 2]
                dt3 = ds3[i % 2]
                # d[c] = |fit[f, i-10+c] - samples[q, i]|, c in [0,20)
                nc.scalar.activation(
                    out=dt3[:, :, 1:21],
                    in_=fit3[:, :, i + PAD - 10 : i + PAD + 10],
                    func=fabs,
                    bias=nsamp[:, i : i + 1],
                    scale=1.0,
                )
                # a[c] = min(prev[c], prev[c+1])
                nc.vector.tensor_tensor(
                    out=a3[:, :, 1:21],
                    in0=rin3[:, :, 1:21],
                    in1=rin3[:, :, 2:22],
                    op=amin,
                )
                # row[c] = min(a[c], carry) + d[c]  (segmented via guards)
                nc.vector.tensor_tensor_scan(
                    out=rout[:, :],
                    data0=a_arr[:, :],
                    data1=dt[:, :],
                    initial=float(LARGE),
                    op0=amin,
                    op1=aadd,
                )

            # dm = cost[255,255] = final row cell c=10 (offset 11 per segment)
            rf3 = rows3[(M - 1) % 2]
            dmc3 = dmc.rearrange("p (s o) -> p s o", o=1)
            nc.vector.tensor_copy(out=dmc3[:, :, 0:1], in_=rf3[:, :, 11:12])
            nc.sync.dma_start(out=dm_out[:, :], in_=dmc[:])

    nc.compile()
    return nc


def _host_inputs(samples: np.ndarray, fit: np.ndarray):
    """Per-core in_maps for run_bass_kernel_spmd."""
    pidx = np.arange(128)
    fidx = (pidx % NCORES)[:, None] * NSEG + np.arange(NSEG)[None, :]  # [128,32]

    fit_pad = np.full((NF, PADF), LARGE, np.float32)
    fit_pad[:, PAD : PAD + N] = fit
    fit_rep = np.ascontiguousarray(fit_pad[fidx].reshape(128, NSEG * PADF))

    a_tmpl = np.full((128, NSEG, SEG), LARGE, np.float32)
    a_tmpl[:, :, 0] = -LARGE
    a_tmpl[:, :, 21] = LARGE
    a_tmpl = a_tmpl.reshape(128, FD)

    d_tmpl = np.full((128, NSEG, SEG), LARGE, np.float32)
    d_tmpl[:, :, 0] = 2 * LARGE
    d_tmpl[:, :, 21] = 2 * LARGE
    d_tmpl = d_tmpl.reshape(128, FD)

    in_maps = []
    for core in range(NCORES):
        qidx = core * QPC + pidx // NCORES  # [128]
        neg_samp = np.ascontiguousarray(-samples[qidx])

        row0 = np.full((128, NSEG, SEG), LARGE, np.float32)
        d0 = np.abs(samples[qidx, 0][:, None, None] - fit[fidx][:, :, 0:11])
        row0[:, :, 11:22] = np.cumsum(d0.astype(np.float32), axis=-1, dtype=np.float32)
        row0 = row0.reshape(128, FD)

        in_maps.append(
            {
                "fit_rep": fit_rep,
                "neg_samp": neg_samp,
                "row0": np.ascontiguousarray(row0),
                "a_tmpl": a_tmpl,
                "d_tmpl": d_tmpl,
            }
        )
    return in_maps


def _assemble_dm(results) -> np.ndarray:
    dm = np.empty((NQ, NF), np.float32)
    for core, res in enumerate(results):
        arr = np.asarray(res["dm_out"], np.float32).reshape(QPC, NCORES, NSEG)
        dm[core * QPC : (core + 1) * QPC] = arr.reshape(QPC, NF)
    return dm


def run_device(samples, fit, **spmd_kwargs):
    """Compile (cached) + run on 8 cores; returns (dm [128,256], BassKernelResults)."""
    if "nc" not in _CACHE:
        _CACHE["nc"] = _build_nc()
    nc = _CACHE["nc"]
    in_maps = _host_inputs(samples, fit)
    res = bass_utils.run_bass_kernel_spmd(
        nc, in_maps, core_ids=list(range(NCORES)), **spmd_kwargs
    )
    return _assemble_dm(res.results), res


def kernel(samples, fit_data, fit_labels):
    samples = np.ascontiguousarray(np.asarray(samples), dtype=np.float32)
    fit = np.ascontiguousarray(np.asarray(fit_data), dtype=np.float32)
    labels = np.asarray(fit_labels)
    dm, _ = run_device(samples, fit)
    knn = np.argmin(dm, axis=1)
    return labels[knn]



# revision 5
# speedup vs baseline: 1.1552x; 1.1552x over previous
"""Banded-DTW 1-NN (KnnDtw) Trainium2 Bass kernel — v4 (DVE dual-chain).

Algorithm
---------
Reference computes, per (query q, fit row f), a Sakoe-Chiba banded DTW
(w=10) over length-256 sequences and returns fit_labels[argmin_f dm[q,f]].

Device mapping: in band coordinates, row i keeps cells x in [1,20] with
j = i-11+x (exactly the reference band [i-10, i+9]); x=0 is a guard that
resets the scan carry between independent (q,f) segments.  The update is
    a[x]   = min(prev[x], prev[x+1])
    row[x] = min(a[x], row[x-1]) + |samples[q,i] - fit[f, i-11+x]|
one `tensor_tensor` min + one `tensor_tensor_scan` (op0=min, op1=add) per
DTW step.  The a/d guard values (-LARGE / +2*LARGE) reset the carry to
LARGE at each segment boundary, letting one scan instruction process many
independent (q,f) pairs per partition.

Band-edge exactness: cell x=20 (j=i+9) has its up-neighbor cost[i-1,i+9]
outside row i-1's band (+inf in the reference), and the shifted min read
for it lands on a guard/pad (LARGE) — so min(prev[20], LARGE) = prev[20]
is exact.  For row 0 (a cumsum), prev[21] >= prev[20] holds, so dropping
the old x=21 slot is also exact.

Schedule: the min+scan recurrence only runs on the DVE (the Pool/GpSimd
Q7 ucode implements no tensor min, and the scan is DVE-only), so the 32
segments are split into two interleaved DVE chains (16+16).  Per step the
program order is TT_A, TT_B, scan_A, scan_B: each instruction's producer
sits two slots back, so its ~95ns semaphore-visibility latency hides
behind the other chain's execution and the DVE runs back-to-back busy.
The Act engine computes all |fit - sample| d-values into 8-step ring
chunks, running ~8 steps ahead so its semaphores are always pre-satisfied.
fit data loads as two column chunks so the bulk of the DMA overlaps the
first ~40 DTW steps.

Sharding: queries split across 8 cores (16 each); per core the 4096 (q,f)
pairs sit on 128 partitions x 32 segments: partition p = q_local*8 + f_hi,
segment s -> f = f_hi*32 + s.

The device returns dm (cost[255,255] per pair); the host does the final
argmin + label gather (trivial, exact).
"""

import numpy as np

import concourse.bass as bass
import concourse.bacc as bacc
import concourse.mybir as mybir
from concourse.tile import TileContext
from concourse import bass_utils

# Problem shapes (hardcoded per harness contract)
NQ, M = 128, 256      # samples
NF, N = 256, 256      # fit_data
NCORES = 8
QPC = NQ // NCORES    # 16 queries per core
SEG = 21              # 1 guard + 20 band cells; j = i-11+x for x in [1,20]
NSEG = 32             # segments (f_lo values) per partition
CA = 16               # segments on DVE chain A
CB = NSEG - CA        # segments on DVE chain B
FD = NSEG * SEG
PAD = 10              # fit col = j + PAD
PADF = N + 19         # 275: cols j in [-10, 264]
FIT_SPLIT = 60        # fit chunk A covers cols [0, 60); B covers [40, 275)
ISPLIT = 40           # steps i <= ISPLIT read chunk A; i > ISPLIT read B
DC = 8                # d-ring chunk size (steps per chunk, 2 chunks)
LARGE = np.float32(1e15)
F32 = mybir.dt.float32

_CACHE: dict = {}


def _build_nc() -> bass.Bass:
    nc = bacc.Bacc(
        "TRN2", target_bir_lowering=False, debug=False, num_devices=NCORES
    )

    BCOLS = PADF - FIT_SPLIT + 20  # chunk B cols [40, 275) of the padded fit

    fita_in = nc.dram_tensor("fit_a", [128, NSEG * FIT_SPLIT], F32, kind="ExternalInput")
    fitb_in = nc.dram_tensor("fit_b", [128, NSEG * BCOLS], F32, kind="ExternalInput")
    nsamp_in = nc.dram_tensor("neg_samp", [128, M], F32, kind="ExternalInput")
    row0_in = nc.dram_tensor("row0_full", [128, FD + 4], F32, kind="ExternalInput")
    dm_out = nc.dram_tensor("dm_out", [128, NSEG], F32, kind="ExternalOutput")

    amin = mybir.AluOpType.min
    aadd = mybir.AluOpType.add
    fabs = mybir.ActivationFunctionType.Abs

    chains = [("a", 0, CA), ("b", CA, CB)]  # (name, seg_lo, nsegs)

    with TileContext(nc) as tc:
        with tc.tile_pool(name="main", bufs=1) as pool:
            fita = pool.tile([128, NSEG * FIT_SPLIT], F32)
            fitb = pool.tile([128, NSEG * BCOLS], F32)
            nsamp = pool.tile([128, M], F32)
            rows = {}
            aas = {}
            for name, _lo, ns in chains:
                fdc = ns * SEG
                rows[name] = [
                    pool.tile([128, fdc + 2], F32, name=f"row_{name}_a"),
                    pool.tile([128, fdc + 2], F32, name=f"row_{name}_b"),
                ]
                aas[name] = pool.tile([128, fdc], F32, name=f"aa_{name}")
            d_c0 = pool.tile([128, DC * FD], F32)
            d_c1 = pool.tile([128, DC * FD], F32)
            dmc = pool.tile([128, NSEG], F32)

            # Startup DMAs: needs of early steps first; the big fit chunk B
            # last so it overlaps the first ~40 DTW steps.
            nc.sync.dma_start(out=fita[:], in_=fita_in[:, :])
            nc.sync.dma_start(out=nsamp[:], in_=nsamp_in[:, :])
            off = 0
            for name, _lo, ns in chains:
                fdc = ns * SEG
                nc.sync.dma_start(
                    out=rows[name][0][:], in_=row0_in[:, off : off + fdc + 2]
                )
                off += fdc + 2
            nc.sync.dma_start(out=fitb[:], in_=fitb_in[:, :])

            # Templates via memset: a-guards (-LARGE), d-ring guards
            # (+2*LARGE), trailing pads of the write-side row buffers.
            for name, _lo, ns in chains:
                nc.vector.memset(aas[name][:], -LARGE)
                nc.vector.memset(rows[name][1][:, ns * SEG : ns * SEG + 2], LARGE)
            for dch in (d_c0, d_c1):
                d4 = dch.rearrange("p (k s c) -> p k s c", k=DC, c=SEG)
                nc.vector.memset(d4[:, :, :, 0:1], 2 * LARGE)

            fita3 = fita.rearrange("p (s c) -> p s c", c=FIT_SPLIT)
            fitb3 = fitb.rearrange("p (s c) -> p s c", c=BCOLS)
            dchunks = [d_c0, d_c1]
            d4s = [d.rearrange("p (k s c) -> p k s c", k=DC, c=SEG) for d in dchunks]

            for i in range(1, M):
                k = (i - 1) % DC
                dch = dchunks[((i - 1) // DC) % 2]
                d4 = d4s[((i - 1) // DC) % 2]
                # d[x] = |fit[f, i-11+x] - samples[q, i]|, x in [1,21)
                if i <= ISPLIT:
                    fwin = fita3[:, :, i : i + 20]
                else:
                    fwin = fitb3[:, :, i - FIT_SPLIT + 20 : i - FIT_SPLIT + 40]
                nc.scalar.activation(
                    out=d4[:, k, :, 1:21],
                    in_=fwin,
                    func=fabs,
                    bias=nsamp[:, i : i + 1],
                    scale=1.0,
                )

                # two interleaved DVE chains: TT_A, TT_B, scan_A, scan_B
                for name, _lo, ns in chains:
                    fdc = ns * SEG
                    rin = rows[name][(i - 1) % 2]
                    rin3 = rin[:, 0:fdc].rearrange("p (s c) -> p s c", c=SEG)
                    rsh3 = rin[:, 2 : fdc + 2].rearrange("p (s c) -> p s c", c=SEG)
                    aa3 = aas[name].rearrange("p (s c) -> p s c", c=SEG)
                    nc.vector.tensor_tensor(
                        out=aa3[:, :, 1:21],
                        in0=rin3[:, :, 1:21],
                        in1=rsh3[:, :, 0:20],
                        op=amin,
                    )
                for name, lo, ns in chains:
                    fdc = ns * SEG
                    rout = rows[name][i % 2]
                    nc.vector.tensor_tensor_scan(
                        out=rout[:, 0:fdc],
                        data0=aas[name][:, :],
                        data1=dch[:, k * FD + lo * SEG : k * FD + lo * SEG + fdc],
                        initial=float(LARGE),
                        op0=amin,
                        op1=aadd,
                    )

            # dm = cost[255,255] = final row cell x=11 per segment
            dmc3 = dmc.rearrange("p (s o) -> p s o", o=1)
            for name, lo, ns in chains:
                fdc = ns * SEG
                f3 = rows[name][(M - 1) % 2][:, 0:fdc].rearrange(
                    "p (s c) -> p s c", c=SEG
                )
                nc.vector.tensor_copy(out=dmc3[:, lo : lo + ns, 0:1], in_=f3[:, :, 11:12])
            nc.sync.dma_start(out=dm_out[:, :], in_=dmc[:])

    nc.compile()
    return nc


def _host_inputs(samples: np.ndarray, fit: np.ndarray):
    """Per-core in_maps for run_bass_kernel_spmd."""
    pidx = np.arange(128)
    fidx = (pidx % NCORES)[:, None] * NSEG + np.arange(NSEG)[None, :]  # [128,32]

    fit_pad = np.full((NF, PADF), LARGE, np.float32)
    fit_pad[:, PAD : PAD + N] = fit
    fit_rep = fit_pad[fidx]  # [128, 32, PADF]
    fita = np.ascontiguousarray(fit_rep[:, :, :FIT_SPLIT].reshape(128, -1))
    fitb = np.ascontiguousarray(fit_rep[:, :, FIT_SPLIT - 20 :].reshape(128, -1))

    in_maps = []
    for core in range(NCORES):
        qidx = core * QPC + pidx // NCORES  # [128]
        neg_samp = np.ascontiguousarray(-samples[qidx])

        row0 = np.full((128, NSEG, SEG), LARGE, np.float32)
        d0 = np.abs(samples[qidx, 0][:, None, None] - fit[fidx][:, :, 0:10])
        row0[:, :, 11:21] = np.cumsum(d0.astype(np.float32), axis=-1, dtype=np.float32)
        row0 = row0.reshape(128, NSEG * SEG)
        # concatenated per-chain row0 blocks, each with 2 trailing pads
        row0full = np.full((128, NSEG * SEG + 4), LARGE, np.float32)
        off = 0
        for lo, ns in ((0, CA), (CA, CB)):
            fdc = ns * SEG
            row0full[:, off : off + fdc] = row0[:, lo * SEG : lo * SEG + fdc]
            off += fdc + 2

        in_maps.append(
            {
                "fit_a": fita,
                "fit_b": fitb,
                "neg_samp": neg_samp,
                "row0_full": np.ascontiguousarray(row0full),
            }
        )
    return in_maps


def _assemble_dm(results) -> np.ndarray:
    dm = np.empty((NQ, NF), np.float32)
    for core, res in enumerate(results):
        arr = np.asarray(res["dm_out"], np.float32).reshape(QPC, NCORES, NSEG)
        dm[core * QPC : (core + 1) * QPC] = arr.reshape(QPC, NF)
    return dm


def run_device(samples, fit, **spmd_kwargs):
    """Compile (cached) + run on 8 cores; returns (dm [128,256], BassKernelResults)."""
    if "nc" not in _CACHE:
        _CACHE["nc"] = _build_nc()
    nc = _CACHE["nc"]
    in_maps = _host_inputs(samples, fit)
    res = bass_utils.run_bass_kernel_spmd(
        nc, in_maps, core_ids=list(range(NCORES)), **spmd_kwargs
    )
    return _assemble_dm(res.results), res


def kernel(samples, fit_data, fit_labels):
    samples = np.ascontiguousarray(np.asarray(samples), dtype=np.float32)
    fit = np.ascontiguousarray(np.asarray(fit_data), dtype=np.float32)
    labels = np.asarray(fit_labels)
    dm, _ = run_device(samples, fit)
    knn = np.argmin(dm, axis=1)
    return labels[knn]


# revision 7
# speedup vs baseline: 1.1818x; 1.0230x over previous
"""Banded-DTW 1-NN (KnnDtw) Trainium2 Bass kernel — v6 (DVE dual-chain,
in-place neighbor-min).

Algorithm
---------
Reference computes, per (query q, fit row f), a Sakoe-Chiba banded DTW
(w=10) over length-256 sequences and returns fit_labels[argmin_f dm[q,f]].

Device mapping: in band coordinates, row i keeps cells x in [1,20] with
j = i-11+x (exactly the reference band [i-10, i+9]); x=0 is a guard that
resets the scan carry between independent (q,f) segments.  The update is
    a[x]   = min(prev[x], prev[x+1])
    row[x] = min(a[x], row[x-1]) + |samples[q,i] - fit[f, i-11+x]|
one `tensor_tensor` min + one `tensor_tensor_scan` (op0=min, op1=add) per
DTW step.

The neighbor-min runs IN PLACE on the previous row tile over x in [1,19]
(reads of r[x+1] stay ahead of writes of r[x] in the DVE pipeline, which
was verified on hardware):
  - cell x=20 keeps prev[20], which IS the correct a[20]: its up-neighbor
    cost[i-1, i+9] lies outside row i-1's band (+inf in the reference),
    and for row 0 (a cumsum) prev[21] >= prev[20] holds;
  - the x=0 guard keeps its LARGE scan output, so the following scan's
    carry still resets at every segment boundary (min(LARGE, carry) +
    2*LARGE >= 2*LARGE).
The scan then uses the modified row tile directly as data0 — no separate
a-array, and the per-step tensor_tensor shrinks to 19 cells/segment.

Schedule: the recurrence only runs on the DVE (the Pool/GpSimd Q7 ucode
implements no tensor min, and the scan is DVE-only), so the 32 segments
split into two interleaved DVE chains (16+16).  Per step the program
order is TT_A, TT_B, scan_A, scan_B: each instruction's producer sits two
slots back, so its ~95ns semaphore-visibility latency hides behind the
other chain's execution and the DVE runs back-to-back busy.  The Act
engine computes all |fit - sample| d-values into 8-step ring chunks,
running ahead so its semaphores are pre-satisfied.  fit data loads as two
column chunks so the bulk of the DMA overlaps the first ~19 DTW steps.

Sharding: queries split across 8 cores (16 each); per core the 4096 (q,f)
pairs sit on 128 partitions x 32 segments: partition p = q_local*8 + f_hi,
segment s -> f = f_hi*32 + s.

The device returns dm (cost[255,255] per pair); the host does the final
argmin + label gather (trivial, exact).
"""

import numpy as np

import concourse.bass as bass
import concourse.bacc as bacc
import concourse.mybir as mybir
from concourse.tile import TileContext
from concourse import bass_utils

# Problem shapes (hardcoded per harness contract)
NQ, M = 128, 256      # samples
NF, N = 256, 256      # fit_data
NCORES = 8
QPC = NQ // NCORES    # 16 queries per core
SEG = 21              # 1 guard + 20 band cells; j = i-11+x for x in [1,20]
NSEG = 32             # segments (f_lo values) per partition
CA = 16               # segments on DVE chain A
CB = NSEG - CA        # segments on DVE chain B
FD = NSEG * SEG
PAD = 10              # fit col = j + PAD
PADF = N + 19         # 275: cols j in [-10, 264]
FIT_SPLIT = 40        # fit chunk A covers cols [0, 40); B covers [20, 275)
ISPLIT = 19           # steps i <= ISPLIT read chunk A; i > ISPLIT read B
DC = 8                # d-ring chunk size (steps per chunk, 2 chunks)
LARGE = np.float32(1e15)
F32 = mybir.dt.float32

_CACHE: dict = {}


def _build_nc() -> bass.Bass:
    nc = bacc.Bacc(
        "TRN2", target_bir_lowering=False, debug=False, num_devices=NCORES
    )

    BCOLS = PADF - FIT_SPLIT + 20  # chunk B cols [20, 275) of the padded fit

    fita_in = nc.dram_tensor("fit_a", [128, NSEG * FIT_SPLIT], F32, kind="ExternalInput")
    fitb_in = nc.dram_tensor("fit_b", [128, NSEG * BCOLS], F32, kind="ExternalInput")
    nsamp_in = nc.dram_tensor("neg_samp", [128, M], F32, kind="ExternalInput")
    row0_in = nc.dram_tensor("row0_full", [128, FD + 4], F32, kind="ExternalInput")
    dm_out = nc.dram_tensor("dm_out", [128, NSEG], F32, kind="ExternalOutput")

    amin = mybir.AluOpType.min
    aadd = mybir.AluOpType.add
    fabs = mybir.ActivationFunctionType.Abs

    chains = [("a", 0, CA), ("b", CA, CB)]  # (name, seg_lo, nsegs)

    with TileContext(nc) as tc:
        with tc.tile_pool(name="main", bufs=1) as pool:
            fita = pool.tile([128, NSEG * FIT_SPLIT], F32)
            fitb = pool.tile([128, NSEG * BCOLS], F32)
            nsamp = pool.tile([128, M], F32)
            rows = {}
            for name, _lo, ns in chains:
                fdc = ns * SEG
                rows[name] = [
                    pool.tile([128, fdc + 2], F32, name=f"row_{name}_a"),
                    pool.tile([128, fdc + 2], F32, name=f"row_{name}_b"),
                ]
            d_c0 = pool.tile([128, DC * FD], F32)
            d_c1 = pool.tile([128, DC * FD], F32)
            dmc = pool.tile([128, NSEG], F32)

            # Startup DMAs: needs of early steps first; the big fit chunk B
            # last so it overlaps the first ~19 DTW steps.
            nc.sync.dma_start(out=fita[:], in_=fita_in[:, :])
            nc.sync.dma_start(out=nsamp[:], in_=nsamp_in[:, :])
            off = 0
            for name, _lo, ns in chains:
                fdc = ns * SEG
                nc.sync.dma_start(
                    out=rows[name][0][:], in_=row0_in[:, off : off + fdc + 2]
                )
                off += fdc + 2
            nc.sync.dma_start(out=fitb[:], in_=fitb_in[:, :])

            # d-ring guards (+2*LARGE) via Pool memsets, off the DVE's
            # startup path.  (Row-buffer guards come from the scan itself:
            # the write-side row buffer is fully written by the step-1 scan
            # before any read of it, and the trailing pads are never read.)
            for dch in (d_c0, d_c1):
                dg3 = dch.rearrange("p (g c) -> p g c", c=SEG)
                nc.gpsimd.memset(dg3[:, :, 0:1], 2 * LARGE)

            fita3 = fita.rearrange("p (s c) -> p s c", c=FIT_SPLIT)
            fitb3 = fitb.rearrange("p (s c) -> p s c", c=BCOLS)
            dchunks = [d_c0, d_c1]
            d4s = [d.rearrange("p (k s c) -> p k s c", k=DC, c=SEG) for d in dchunks]

            for i in range(1, M):
                k = (i - 1) % DC
                dch = dchunks[((i - 1) // DC) % 2]
                d4 = d4s[((i - 1) // DC) % 2]
                # d[x] = |fit[f, i-11+x] - samples[q, i]|, x in [1,21)
                if i <= ISPLIT:
                    fwin = fita3[:, :, i : i + 20]
                else:
                    fwin = fitb3[:, :, i - FIT_SPLIT + 20 : i - FIT_SPLIT + 40]
                nc.scalar.activation(
                    out=d4[:, k, :, 1:21],
                    in_=fwin,
                    func=fabs,
                    bias=nsamp[:, i : i + 1],
                    scale=1.0,
                )

                # two interleaved DVE chains: TT_A, TT_B, scan_A, scan_B;
                # the TT updates the prev row in place (a-values in x[1,19],
                # a[20]=prev[20], guards keep LARGE).
                for name, _lo, ns in chains:
                    fdc = ns * SEG
                    rin = rows[name][(i - 1) % 2]
                    rin3 = rin[:, 0:fdc].rearrange("p (s c) -> p s c", c=SEG)
                    rsh3 = rin[:, 2 : fdc + 2].rearrange("p (s c) -> p s c", c=SEG)
                    nc.vector.tensor_tensor(
                        out=rin3[:, :, 1:20],
                        in0=rin3[:, :, 1:20],
                        in1=rsh3[:, :, 0:19],
                        op=amin,
                    )
                for name, lo, ns in chains:
                    fdc = ns * SEG
                    rin = rows[name][(i - 1) % 2]
                    rout = rows[name][i % 2]
                    nc.vector.tensor_tensor_scan(
                        out=rout[:, 0:fdc],
                        data0=rin[:, 0:fdc],
                        data1=dch[:, k * FD + lo * SEG : k * FD + lo * SEG + fdc],
                        initial=float(LARGE),
                        op0=amin,
                        op1=aadd,
                    )

            # dm = cost[255,255] = final row cell x=11 per segment
            dmc3 = dmc.rearrange("p (s o) -> p s o", o=1)
            for name, lo, ns in chains:
                fdc = ns * SEG
                f3 = rows[name][(M - 1) % 2][:, 0:fdc].rearrange(
                    "p (s c) -> p s c", c=SEG
                )
                nc.vector.tensor_copy(out=dmc3[:, lo : lo + ns, 0:1], in_=f3[:, :, 11:12])
            nc.sync.dma_start(out=dm_out[:, :], in_=dmc[:])

    nc.compile()
    return nc


def _host_inputs(samples: np.ndarray, fit: np.ndarray):
    """Per-core in_maps for run_bass_kernel_spmd."""
    pidx = np.arange(128)
    fidx = (pidx % NCORES)[:, None] * NSEG + np.arange(NSEG)[None, :]  # [128,32]

    fit_pad = np.full((NF, PADF), LARGE, np.float32)
    fit_pad[:, PAD : PAD + N] = fit
    fit_rep = fit_pad[fidx]  # [128, 32, PADF]
    fita = np.ascontiguousarray(fit_rep[:, :, :FIT_SPLIT].reshape(128, -1))
    fitb = np.ascontiguousarray(fit_rep[:, :, FIT_SPLIT - 20 :].reshape(128, -1))

    in_maps = []
    for core in range(NCORES):
        qidx = core * QPC + pidx // NCORES  # [128]
        neg_samp = np.ascontiguousarray(-samples[qidx])

        row0 = np.full((128, NSEG, SEG), LARGE, np.float32)
        d0 = np.abs(samples[qidx, 0][:, None, None] - fit[fidx][:, :, 0:10])
        row0[:, :, 11:21] = np.cumsum(d0.astype(np.float32), axis=-1, dtype=np.float32)
        row0 = row0.reshape(128, NSEG * SEG)
        # concatenated per-chain row0 blocks, each with 2 trailing pads
        row0full = np.full((128, NSEG * SEG + 4), LARGE, np.float32)
        off = 0
        for lo, ns in ((0, CA), (CA, CB)):
            fdc = ns * SEG
            row0full[:, off : off + fdc] = row0[:, lo * SEG : lo * SEG + fdc]
            off += fdc + 2

        in_maps.append(
            {
                "fit_a": fita,
                "fit_b": fitb,
                "neg_samp": neg_samp,
                "row0_full": np.ascontiguousarray(row0full),
            }
        )
    return in_maps


def _assemble_dm(results) -> np.ndarray:
    dm = np.empty((NQ, NF), np.float32)
    for core, res in enumerate(results):
        arr = np.asarray(res["dm_out"], np.float32).reshape(QPC, NCORES, NSEG)
        dm[core * QPC : (core + 1) * QPC] = arr.reshape(QPC, NF)
    return dm


def run_device(samples, fit, **spmd_kwargs):
    """Compile (cached) + run on 8 cores; returns (dm [128,256], BassKernelResults)."""
    if "nc" not in _CACHE:
        _CACHE["nc"] = _build_nc()
    nc = _CACHE["nc"]
    in_maps = _host_inputs(samples, fit)
    res = bass_utils.run_bass_kernel_spmd(
        nc, in_maps, core_ids=list(range(NCORES)), **spmd_kwargs
    )
    return _assemble_dm(res.results), res


def kernel(samples, fit_data, fit_labels):
    samples = np.ascontiguousarray(np.asarray(samples), dtype=np.float32)
    fit = np.ascontiguousarray(np.asarray(fit_data), dtype=np.float32)
    labels = np.asarray(fit_labels)
    dm, _ = run_device(samples, fit)
    knn = np.argmin(dm, axis=1)
    return labels[knn]


# revision 8
# speedup vs baseline: 1.1831x; 1.0011x over previous
"""Banded-DTW 1-NN (KnnDtw) Trainium2 Bass kernel — v6 (DVE dual-chain,
in-place neighbor-min).

Algorithm
---------
Reference computes, per (query q, fit row f), a Sakoe-Chiba banded DTW
(w=10) over length-256 sequences and returns fit_labels[argmin_f dm[q,f]].

Device mapping: in band coordinates, row i keeps cells x in [1,20] with
j = i-11+x (exactly the reference band [i-10, i+9]); x=0 is a guard that
resets the scan carry between independent (q,f) segments.  The update is
    a[x]   = min(prev[x], prev[x+1])
    row[x] = min(a[x], row[x-1]) + |samples[q,i] - fit[f, i-11+x]|
one `tensor_tensor` min + one `tensor_tensor_scan` (op0=min, op1=add) per
DTW step.

The neighbor-min runs IN PLACE on the previous row tile over x in [1,19]
(reads of r[x+1] stay ahead of writes of r[x] in the DVE pipeline, which
was verified on hardware):
  - cell x=20 keeps prev[20], which IS the correct a[20]: its up-neighbor
    cost[i-1, i+9] lies outside row i-1's band (+inf in the reference),
    and for row 0 (a cumsum) prev[21] >= prev[20] holds;
  - the x=0 guard keeps its LARGE scan output, so the following scan's
    carry still resets at every segment boundary (min(LARGE, carry) +
    2*LARGE >= 2*LARGE).
The scan then uses the modified row tile directly as data0 — no separate
a-array, and the per-step tensor_tensor shrinks to 19 cells/segment.

Schedule: the recurrence only runs on the DVE (the Pool/GpSimd Q7 ucode
implements no tensor min, and the scan is DVE-only), so the 32 segments
split into two interleaved DVE chains (16+16).  Per step the program
order is TT_A, TT_B, scan_A, scan_B: each instruction's producer sits two
slots back, so its ~95ns semaphore-visibility latency hides behind the
other chain's execution and the DVE runs back-to-back busy.  The Act
engine computes all |fit - sample| d-values into 8-step ring chunks,
running ahead so its semaphores are pre-satisfied.  fit data loads as two
column chunks so the bulk of the DMA overlaps the first ~19 DTW steps.

Sharding: queries split across 8 cores (16 each); per core the 4096 (q,f)
pairs sit on 128 partitions x 32 segments: partition p = q_local*8 + f_hi,
segment s -> f = f_hi*32 + s.

The device returns dm (cost[255,255] per pair); the host does the final
argmin + label gather (trivial, exact).
"""

import numpy as np

import concourse.bass as bass
import concourse.bacc as bacc
import concourse.mybir as mybir
from concourse.tile import TileContext
from concourse import bass_utils

# Problem shapes (hardcoded per harness contract)
NQ, M = 128, 256      # samples
NF, N = 256, 256      # fit_data
NCORES = 8
QPC = NQ // NCORES    # 16 queries per core
SEG = 21              # 1 guard + 20 band cells; j = i-11+x for x in [1,20]
NSEG = 32             # segments (f_lo values) per partition
CA = 16               # segments on DVE chain A
CB = NSEG - CA        # segments on DVE chain B
FD = NSEG * SEG
PAD = 10              # fit col = j + PAD
PADF = N + 19         # 275: cols j in [-10, 264]
FIT_SPLIT = 30        # fit chunk A covers cols [0, 30); B covers [10, 275)
ISPLIT = 9            # steps i <= ISPLIT read chunk A; i > ISPLIT read B
DC = 8                # d-ring chunk size (steps per chunk, 2 chunks)
LARGE = np.float32(1e15)
F32 = mybir.dt.float32

_CACHE: dict = {}


def _build_nc() -> bass.Bass:
    nc = bacc.Bacc(
        "TRN2", target_bir_lowering=False, debug=False, num_devices=NCORES
    )

    BCOLS = PADF - FIT_SPLIT + 20  # chunk B cols [20, 275) of the padded fit

    fita_in = nc.dram_tensor("fit_a", [128, NSEG * FIT_SPLIT], F32, kind="ExternalInput")
    fitb_in = nc.dram_tensor("fit_b", [128, NSEG * BCOLS], F32, kind="ExternalInput")
    nsamp_in = nc.dram_tensor("neg_samp", [128, M], F32, kind="ExternalInput")
    row0_in = nc.dram_tensor("row0_full", [128, FD + 4], F32, kind="ExternalInput")
    dm_out = nc.dram_tensor("dm_out", [128, NSEG], F32, kind="ExternalOutput")

    amin = mybir.AluOpType.min
    aadd = mybir.AluOpType.add
    fabs = mybir.ActivationFunctionType.Abs

    chains = [("a", 0, CA), ("b", CA, CB)]  # (name, seg_lo, nsegs)

    with TileContext(nc) as tc:
        with tc.tile_pool(name="main", bufs=1) as pool:
            fita = pool.tile([128, NSEG * FIT_SPLIT], F32)
            fitb = pool.tile([128, NSEG * BCOLS], F32)
            nsamp = pool.tile([128, M], F32)
            rows = {}
            for name, _lo, ns in chains:
                fdc = ns * SEG
                rows[name] = [
                    pool.tile([128, fdc + 2], F32, name=f"row_{name}_a"),
                    pool.tile([128, fdc + 2], F32, name=f"row_{name}_b"),
                ]
            d_c0 = pool.tile([128, DC * FD], F32)
            d_c1 = pool.tile([128, DC * FD], F32)
            dmc = pool.tile([128, NSEG], F32)

            # Startup DMAs: needs of early steps first; the big fit chunk B
            # last so it overlaps the first ~19 DTW steps.
            nc.sync.dma_start(out=fita[:], in_=fita_in[:, :])
            nc.sync.dma_start(out=nsamp[:], in_=nsamp_in[:, :])
            off = 0
            for name, _lo, ns in chains:
                fdc = ns * SEG
                nc.sync.dma_start(
                    out=rows[name][0][:], in_=row0_in[:, off : off + fdc + 2]
                )
                off += fdc + 2
            nc.sync.dma_start(out=fitb[:], in_=fitb_in[:, :])

            # d-ring guards (+2*LARGE) via Pool memsets, off the DVE's
            # startup path.  (Row-buffer guards come from the scan itself:
            # the write-side row buffer is fully written by the step-1 scan
            # before any read of it, and the trailing pads are never read.)
            for dch in (d_c0, d_c1):
                dg3 = dch.rearrange("p (g c) -> p g c", c=SEG)
                nc.gpsimd.memset(dg3[:, :, 0:1], 2 * LARGE)

            fita3 = fita.rearrange("p (s c) -> p s c", c=FIT_SPLIT)
            fitb3 = fitb.rearrange("p (s c) -> p s c", c=BCOLS)
            dchunks = [d_c0, d_c1]
            d4s = [d.rearrange("p (k s c) -> p k s c", k=DC, c=SEG) for d in dchunks]

            for i in range(1, M):
                k = (i - 1) % DC
                dch = dchunks[((i - 1) // DC) % 2]
                d4 = d4s[((i - 1) // DC) % 2]
                # d[x] = |fit[f, i-11+x] - samples[q, i]|, x in [1,21)
                if i <= ISPLIT:
                    fwin = fita3[:, :, i : i + 20]
                else:
                    fwin = fitb3[:, :, i - FIT_SPLIT + 20 : i - FIT_SPLIT + 40]
                nc.scalar.activation(
                    out=d4[:, k, :, 1:21],
                    in_=fwin,
                    func=fabs,
                    bias=nsamp[:, i : i + 1],
                    scale=1.0,
                )

                # two interleaved DVE chains: TT_A, TT_B, scan_A, scan_B;
                # the TT updates the prev row in place (a-values in x[1,19],
                # a[20]=prev[20], guards keep LARGE).
                for name, _lo, ns in chains:
                    fdc = ns * SEG
                    rin = rows[name][(i - 1) % 2]
                    rin3 = rin[:, 0:fdc].rearrange("p (s c) -> p s c", c=SEG)
                    rsh3 = rin[:, 2 : fdc + 2].rearrange("p (s c) -> p s c", c=SEG)
                    nc.vector.tensor_tensor(
                        out=rin3[:, :, 1:20],
                        in0=rin3[:, :, 1:20],
                        in1=rsh3[:, :, 0:19],
                        op=amin,
                    )
                for name, lo, ns in chains:
                    fdc = ns * SEG
                    rin = rows[name][(i - 1) % 2]
                    rout = rows[name][i % 2]
                    nc.vector.tensor_tensor_scan(
                        out=rout[:, 0:fdc],
                        data0=rin[:, 0:fdc],
                        data1=dch[:, k * FD + lo * SEG : k * FD + lo * SEG + fdc],
                        initial=float(LARGE),
                        op0=amin,
                        op1=aadd,
                    )

            # dm = cost[255,255] = final row cell x=11 per segment
            dmc3 = dmc.rearrange("p (s o) -> p s o", o=1)
            for name, lo, ns in chains:
                fdc = ns * SEG
                f3 = rows[name][(M - 1) % 2][:, 0:fdc].rearrange(
                    "p (s c) -> p s c", c=SEG
                )
                nc.vector.tensor_copy(out=dmc3[:, lo : lo + ns, 0:1], in_=f3[:, :, 11:12])
            nc.sync.dma_start(out=dm_out[:, :], in_=dmc[:])

    nc.compile()
    return nc


def _host_inputs(samples: np.ndarray, fit: np.ndarray):
    """Per-core in_maps for run_bass_kernel_spmd."""
    pidx = np.arange(128)
    fidx = (pidx % NCORES)[:, None] * NSEG + np.arange(NSEG)[None, :]  # [128,32]

    fit_pad = np.full((NF, PADF), LARGE, np.float32)
    fit_pad[:, PAD : PAD + N] = fit
    fit_rep = fit_pad[fidx]  # [128, 32, PADF]
    fita = np.ascontiguousarray(fit_rep[:, :, :FIT_SPLIT].reshape(128, -1))
    fitb = np.ascontiguousarray(fit_rep[:, :, FIT_SPLIT - 20 :].reshape(128, -1))

    in_maps = []
    for core in range(NCORES):
        qidx = core * QPC + pidx // NCORES  # [128]
        neg_samp = np.ascontiguousarray(-samples[qidx])

        row0 = np.full((128, NSEG, SEG), LARGE, np.float32)
        d0 = np.abs(samples[qidx, 0][:, None, None] - fit[fidx][:, :, 0:10])
        row0[:, :, 11:21] = np.cumsum(d0.astype(np.float32), axis=-1, dtype=np.float32)
        row0 = row0.reshape(128, NSEG * SEG)
        # concatenated per-chain row0 blocks, each with 2 trailing pads
        row0full = np.full((128, NSEG * SEG + 4), LARGE, np.float32)
        off = 0
        for lo, ns in ((0, CA), (CA, CB)):
            fdc = ns * SEG
            row0full[:, off : off + fdc] = row0[:, lo * SEG : lo * SEG + fdc]
            off += fdc + 2

        in_maps.append(
            {
                "fit_a": fita,
                "fit_b": fitb,
                "neg_samp": neg_samp,
                "row0_full": np.ascontiguousarray(row0full),
            }
        )
    return in_maps


def _assemble_dm(results) -> np.ndarray:
    dm = np.empty((NQ, NF), np.float32)
    for core, res in enumerate(results):
        arr = np.asarray(res["dm_out"], np.float32).reshape(QPC, NCORES, NSEG)
        dm[core * QPC : (core + 1) * QPC] = arr.reshape(QPC, NF)
    return dm


def run_device(samples, fit, **spmd_kwargs):
    """Compile (cached) + run on 8 cores; returns (dm [128,256], BassKernelResults)."""
    if "nc" not in _CACHE:
        _CACHE["nc"] = _build_nc()
    nc = _CACHE["nc"]
    in_maps = _host_inputs(samples, fit)
    res = bass_utils.run_bass_kernel_spmd(
        nc, in_maps, core_ids=list(range(NCORES)), **spmd_kwargs
    )
    return _assemble_dm(res.results), res


def kernel(samples, fit_data, fit_labels):
    samples = np.ascontiguousarray(np.asarray(samples), dtype=np.float32)
    fit = np.ascontiguousarray(np.asarray(fit_data), dtype=np.float32)
    labels = np.asarray(fit_labels)
    dm, _ = run_device(samples, fit)
    knn = np.argmin(dm, axis=1)
    return labels[knn]


# revision 11
# speedup vs baseline: 1.1914x; 1.0070x over previous
"""Banded-DTW 1-NN (KnnDtw) Trainium2 Bass kernel — v6 (DVE dual-chain,
in-place neighbor-min).

Algorithm
---------
Reference computes, per (query q, fit row f), a Sakoe-Chiba banded DTW
(w=10) over length-256 sequences and returns fit_labels[argmin_f dm[q,f]].

Device mapping: in band coordinates, row i keeps cells x in [1,20] with
j = i-11+x (exactly the reference band [i-10, i+9]); x=0 is a guard that
resets the scan carry between independent (q,f) segments.  The update is
    a[x]   = min(prev[x], prev[x+1])
    row[x] = min(a[x], row[x-1]) + |samples[q,i] - fit[f, i-11+x]|
one `tensor_tensor` min + one `tensor_tensor_scan` (op0=min, op1=add) per
DTW step.

The neighbor-min runs IN PLACE on the previous row tile over x in [1,19]
(reads of r[x+1] stay ahead of writes of r[x] in the DVE pipeline, which
was verified on hardware):
  - cell x=20 keeps prev[20], which IS the correct a[20]: its up-neighbor
    cost[i-1, i+9] lies outside row i-1's band (+inf in the reference),
    and for row 0 (a cumsum) prev[21] >= prev[20] holds;
  - the x=0 guard keeps its LARGE scan output, so the following scan's
    carry still resets at every segment boundary (min(LARGE, carry) +
    2*LARGE >= 2*LARGE).
The scan then uses the modified row tile directly as data0 — no separate
a-array, and the per-step tensor_tensor shrinks to 19 cells/segment.

Schedule: the recurrence only runs on the DVE (the Pool/GpSimd Q7 ucode
implements no tensor min, and the scan is DVE-only), so the 32 segments
split into two interleaved DVE chains (16+16).  Per step the program
order is TT_A, TT_B, scan_A, scan_B: each instruction's producer sits two
slots back, so its ~95ns semaphore-visibility latency hides behind the
other chain's execution and the DVE runs back-to-back busy.  The Act
engine computes all |fit - sample| d-values into 8-step ring chunks,
running ahead so its semaphores are pre-satisfied.  fit data loads as two
column chunks so the bulk of the DMA overlaps the first ~19 DTW steps.

Sharding: queries split across 8 cores (16 each); per core the 4096 (q,f)
pairs sit on 128 partitions x 32 segments: partition p = q_local*8 + f_hi,
segment s -> f = f_hi*32 + s.

The device returns dm (cost[255,255] per pair); the host does the final
argmin + label gather (trivial, exact).
"""

import numpy as np

import concourse.bass as bass
import concourse.bacc as bacc
import concourse.mybir as mybir
from concourse.tile import TileContext
from concourse import bass_utils

# Problem shapes (hardcoded per harness contract)
NQ, M = 128, 256      # samples
NF, N = 256, 256      # fit_data
NCORES = 8
QPC = NQ // NCORES    # 16 queries per core
SEG = 21              # 1 guard + 20 band cells; j = i-11+x for x in [1,20]
NSEG = 32             # segments (f_lo values) per partition
CA = 16               # segments on DVE chain A
CB = NSEG - CA        # segments on DVE chain B
FD = NSEG * SEG
PAD = 10              # fit col = j + PAD
PADF = N + 19         # 275: cols j in [-10, 264]
FIT_SPLIT = 30        # fit chunk A covers cols [0, 30); B covers [10, 275)
ISPLIT = 9            # steps i <= ISPLIT read chunk A; i > ISPLIT read B
DC = 8                # d-ring chunk size (steps per chunk, 2 chunks)
LARGE = np.float32(1e15)
F32 = mybir.dt.float32

_CACHE: dict = {}


def _build_nc() -> bass.Bass:
    nc = bacc.Bacc(
        "TRN2", target_bir_lowering=False, debug=False, num_devices=NCORES
    )

    BCOLS = PADF - FIT_SPLIT + 20  # chunk B cols [20, 275) of the padded fit

    fita_in = nc.dram_tensor("fit_a", [128, NSEG * FIT_SPLIT], F32, kind="ExternalInput")
    fitb_in = nc.dram_tensor("fit_b", [128, NSEG * BCOLS], F32, kind="ExternalInput")
    nsamp_in = nc.dram_tensor("neg_samp", [128, M], F32, kind="ExternalInput")
    row0_in = nc.dram_tensor("row0_full", [128, FD + 4], F32, kind="ExternalInput")
    dfirst_in = nc.dram_tensor("d_first", [128, FD], F32, kind="ExternalInput")
    dm_out = nc.dram_tensor("dm_out", [128, NSEG], F32, kind="ExternalOutput")

    amin = mybir.AluOpType.min
    aadd = mybir.AluOpType.add
    fabs = mybir.ActivationFunctionType.Abs

    chains = [("a", 0, CA), ("b", CA, CB)]  # (name, seg_lo, nsegs)

    with TileContext(nc) as tc:
        with tc.tile_pool(name="main", bufs=1) as pool:
            fita = pool.tile([128, NSEG * FIT_SPLIT], F32)
            fitb = pool.tile([128, NSEG * BCOLS], F32)
            nsamp = pool.tile([128, M], F32)
            rows = {}
            for name, _lo, ns in chains:
                fdc = ns * SEG
                rows[name] = [
                    pool.tile([128, fdc + 2], F32, name=f"row_{name}_a"),
                    pool.tile([128, fdc + 2], F32, name=f"row_{name}_b"),
                ]
            d_c0 = pool.tile([128, DC * FD], F32)
            d_c1 = pool.tile([128, DC * FD], F32)
            dmc = pool.tile([128, NSEG], F32)

            # Startup DMAs, spread over three HWDGE queues so the
            # transfers parallelize: SP takes step-1 d-values (host
            # precomputed), fit chunk A, nsamp, then the big fit chunk B
            # (overlaps the first ~19 DTW steps); DVE and Act each take one
            # row0 block.
            nc.sync.dma_start(out=d_c0[:, 0:FD], in_=dfirst_in[:, :])
            nc.sync.dma_start(out=rows["a"][0][:], in_=row0_in[:, 0 : CA * SEG + 2])
            nc.gpsimd.dma_start(
                out=rows["b"][0][:], in_=row0_in[:, CA * SEG + 2 : FD + 4]
            )
            nc.scalar.dma_start(out=fita[:], in_=fita_in[:, :])
            nc.sync.dma_start(out=nsamp[:], in_=nsamp_in[:, :])
            nc.sync.dma_start(out=fitb[:], in_=fitb_in[:, :])

            # d-ring guards (+2*LARGE) via Pool memsets, off the DVE's
            # startup path.  (Row-buffer guards come from the scan itself:
            # the write-side row buffer is fully written by the step-1 scan
            # before any read of it, and the trailing pads are never read.)
            dg0 = d_c0.rearrange("p (g c) -> p g c", c=SEG)
            nc.gpsimd.memset(dg0[:, NSEG:, 0:1], 2 * LARGE)
            dg1 = d_c1.rearrange("p (g c) -> p g c", c=SEG)
            nc.gpsimd.memset(dg1[:, :, 0:1], 2 * LARGE)

            fita3 = fita.rearrange("p (s c) -> p s c", c=FIT_SPLIT)
            fitb3 = fitb.rearrange("p (s c) -> p s c", c=BCOLS)
            dchunks = [d_c0, d_c1]
            d4s = [d.rearrange("p (k s c) -> p k s c", k=DC, c=SEG) for d in dchunks]

            for i in range(1, M):
                k = (i - 1) % DC
                dch = dchunks[((i - 1) // DC) % 2]
                d4 = d4s[((i - 1) // DC) % 2]
                # d[x] = |fit[f, i-11+x] - samples[q, i]|, x in [1,21)
                # (step 1's d arrives via the d_first DMA)
                if i >= 2:
                    if i <= ISPLIT:
                        fwin = fita3[:, :, i : i + 20]
                    else:
                        fwin = fitb3[:, :, i - FIT_SPLIT + 20 : i - FIT_SPLIT + 40]
                    nc.scalar.activation(
                        out=d4[:, k, :, 1:21],
                        in_=fwin,
                        func=fabs,
                        bias=nsamp[:, i : i + 1],
                        scale=1.0,
                    )

                # two interleaved DVE chains: TT_A, TT_B, scan_A, scan_B;
                # the TT updates the prev row in place (a-values in x[1,19],
                # a[20]=prev[20], guards keep LARGE).
                # band-edge clip: cells with j<0 (early steps) or j>255
                # (late steps) keep their huge prev values — their own d is
                # LARGE, so skipping their neighbor-min is exact.
                xlo = max(1, 11 - i)
                xhi = min(20, 267 - i)
                for name, _lo, ns in chains:
                    fdc = ns * SEG
                    rin = rows[name][(i - 1) % 2]
                    rin3 = rin[:, 0:fdc].rearrange("p (s c) -> p s c", c=SEG)
                    rsh3 = rin[:, 2 : fdc + 2].rearrange("p (s c) -> p s c", c=SEG)
                    nc.vector.tensor_tensor(
                        out=rin3[:, :, xlo:xhi],
                        in0=rin3[:, :, xlo:xhi],
                        in1=rsh3[:, :, xlo - 1 : xhi - 1],
                        op=amin,
                    )
                for name, lo, ns in chains:
                    fdc = ns * SEG
                    rin = rows[name][(i - 1) % 2]
                    rout = rows[name][i % 2]
                    nc.vector.tensor_tensor_scan(
                        out=rout[:, 0:fdc],
                        data0=rin[:, 0:fdc],
                        data1=dch[:, k * FD + lo * SEG : k * FD + lo * SEG + fdc],
                        initial=float(LARGE),
                        op0=amin,
                        op1=aadd,
                    )

            # dm = cost[255,255] = final row cell x=11 per segment
            dmc3 = dmc.rearrange("p (s o) -> p s o", o=1)
            for name, lo, ns in chains:
                fdc = ns * SEG
                f3 = rows[name][(M - 1) % 2][:, 0:fdc].rearrange(
                    "p (s c) -> p s c", c=SEG
                )
                nc.vector.tensor_copy(out=dmc3[:, lo : lo + ns, 0:1], in_=f3[:, :, 11:12])
            nc.sync.dma_start(out=dm_out[:, :], in_=dmc[:])

    nc.compile()
    return nc


def _host_inputs(samples: np.ndarray, fit: np.ndarray):
    """Per-core in_maps for run_bass_kernel_spmd."""
    pidx = np.arange(128)
    fidx = (pidx % NCORES)[:, None] * NSEG + np.arange(NSEG)[None, :]  # [128,32]

    fit_pad = np.full((NF, PADF), LARGE, np.float32)
    fit_pad[:, PAD : PAD + N] = fit
    fit_rep = fit_pad[fidx]  # [128, 32, PADF]
    fita = np.ascontiguousarray(fit_rep[:, :, :FIT_SPLIT].reshape(128, -1))
    fitb = np.ascontiguousarray(fit_rep[:, :, FIT_SPLIT - 20 :].reshape(128, -1))

    in_maps = []
    for core in range(NCORES):
        qidx = core * QPC + pidx // NCORES  # [128]
        neg_samp = np.ascontiguousarray(-samples[qidx])

        row0 = np.full((128, NSEG, SEG), LARGE, np.float32)
        d0 = np.abs(samples[qidx, 0][:, None, None] - fit[fidx][:, :, 0:10])
        row0[:, :, 11:21] = np.cumsum(d0.astype(np.float32), axis=-1, dtype=np.float32)
        row0 = row0.reshape(128, NSEG * SEG)
        # concatenated per-chain row0 blocks, each with 2 trailing pads
        row0full = np.full((128, NSEG * SEG + 4), LARGE, np.float32)
        off = 0
        for lo, ns in ((0, CA), (CA, CB)):
            fdc = ns * SEG
            row0full[:, off : off + fdc] = row0[:, lo * SEG : lo * SEG + fdc]
            off += fdc + 2

        d1 = np.full((128, NSEG, SEG), 2 * LARGE, np.float32)
        d1[:, :, 1:21] = np.abs(fit_rep[:, :, 1:21] - samples[qidx, 1][:, None, None])
        in_maps.append(
            {
                "fit_a": fita,
                "fit_b": fitb,
                "neg_samp": neg_samp,
                "row0_full": np.ascontiguousarray(row0full),
                "d_first": np.ascontiguousarray(d1.reshape(128, NSEG * SEG)),
            }
        )
    return in_maps


def _assemble_dm(results) -> np.ndarray:
    dm = np.empty((NQ, NF), np.float32)
    for core, res in enumerate(results):
        arr = np.asarray(res["dm_out"], np.float32).reshape(QPC, NCORES, NSEG)
        dm[core * QPC : (core + 1) * QPC] = arr.reshape(QPC, NF)
    return dm


def run_device(samples, fit, **spmd_kwargs):
    """Compile (cached) + run on 8 cores; returns (dm [128,256], BassKernelResults)."""
    if "nc" not in _CACHE:
        _CACHE["nc"] = _build_nc()
    nc = _CACHE["nc"]
    in_maps = _host_inputs(samples, fit)
    res = bass_utils.run_bass_kernel_spmd(
        nc, in_maps, core_ids=list(range(NCORES)), **spmd_kwargs
    )
    return _assemble_dm(res.results), res


def kernel(samples, fit_data, fit_labels):
    samples = np.ascontiguousarray(np.asarray(samples), dtype=np.float32)
    fit = np.ascontiguousarray(np.asarray(fit_data), dtype=np.float32)
    labels = np.asarray(fit_labels)
    dm, _ = run_device(samples, fit)
    knn = np.argmin(dm, axis=1)
    return labels[knn]
